# revision 29
# baseline (speedup 1.0000x reference)
"""AttentionRNNCell Trainium2 kernel (v4).

Math (per batch row b):
  et[t]  = V_a . tanh( (h W_a + b_a) + x[t] U_a )        t in [0, TE)
  at     = exp(et);  s = sum(at)
  ctx    = (sum_t at[t] x[t]) / s
  zt     = sigmoid(h W_z + [inp, ctx] C_z + b_z)
  rt     = sigmoid(h W_r + [inp, ctx] C_r + b_r)
  tht    = tanh((rt*h) U_p + [inp, ctx] C_p + b_p)
  ht     = (1-zt)*h + zt*tht
Distribution: data-parallel over batch B=128 across 8 cores (16 rows each).
Host ships x twice in fp8, pre-tiled in both layouts the PE needs
(xnat: t on partitions for ctx; xtr: e on partitions for uxpb), and folds
everything not depending on x_seq into small per-core tensors.

v4 vs v3 (trace-driven):
  - rows 0/1 xtr go first on the gpsimd (SWDGE) ring in half-row DMAs;
    rows 2-15 ship as PAIR tiles in one DMA each (8KB/partition contiguous
    -> 8KB descriptors; the v3 half-row split produced 1KB descriptors and
    the HWDGE ring drained at only ~76GB/s).
  - xtr pair pool bufs=4 (8 rows of lookahead) so the DMA stream runs at
    ring rate instead of being throttled to compute pace by pool WAR deps
    (v3's row 15 data landed at ~100us, stretching the whole kernel).
  - a dummy ACT right after the preamble pulls the one-time ACT table load
    (~2.7us) off the first-tanh critical path.
  - ctx is transposed + staged per PAIR right after its matmul (DVE copy
    of the [2,256] psum, two tiny PE transposes, DVE copies into ctxn) --
    no SBUF->SBUF ctx_rows DMAs, whose ~2.4us completion latency sat on
    the v3 tail.  Normalization by 1/s happens once per half-batch.
  - gate weights / ctxn / rh are bf16 (fast FWL ldweights; v3's fp32 gate
    LDWEIGHTS were 333ns each) and each gate's two u-chunks share one
    [P,2,8] psum tile -> one ACT per gate (3 per half instead of 6 chains).
  - sigmoid = 1/(1+exp(-x)) on DVE keeps the whole kernel inside the one
    exp_and_others ACT table set.
"""

from contextlib import ExitStack

import numpy as np
import ml_dtypes

import concourse.bass as bass
import concourse.mybir as mybir
import concourse.tile as tile

BF16 = ml_dtypes.bfloat16
NPF8 = ml_dtypes.float8_e4m3
F32 = mybir.dt.float32
BF = mybir.dt.bfloat16
F8 = mybir.dt.float8e4
DR = mybir.MatmulPerfMode.DoubleRow
AF = mybir.ActivationFunctionType
AX = mybir.AxisListType
ALU = mybir.AluOpType

B, TE, U, IN_DIM = 128, 2048, 256, 256
N_CORES = 8
BS = B // N_CORES  # 16 batch rows per core
P = 128
EC = U // P  # e-chunks (2)
UC = U // P  # u-chunks (2)
HB = BS // 2  # rows per tail half (8)


def split_multi_waits(nc, max_waits=1):
    """This container's walrus rejects instructions carrying more than one
    sync wait. Hoist extra waits onto standalone same-engine NoOps inserted
    immediately before the offending instruction (semantically identical:
    the engine blocks on each wait in order before executing it)."""
    n_new = 0
    for f in nc.m.functions:
        for blk in f.blocks:
            new_insts = []
            for inst in blk.instructions:
                si = inst.sync_info
                waits = list(si.on_wait) if si and si.on_wait else []
                if len(waits) > max_waits:
                    for w in waits[:-max_waits]:
                        nop = mybir.InstNoOp(
                            name=f"{inst.name}-hw{n_new}", ins=[], outs=[]
                        )
                        nop.engine = inst.engine
                        nop.sync_info = mybir.SyncInfo(on_wait=[w], on_update=[])
                        new_insts.append(nop)
                        n_new += 1
                    si.on_wait = waits[-max_waits:]
                new_insts.append(inst)
            blk.instructions = new_insts
    return n_new


def build_nc(bs=BS, te=TE, split_waits=True):
    tc_n = te // P      # 128-col t-chunks (16)
    th_n = 2            # t halves
    t_half = te // th_n
    tq_n = t_half // P  # 128-col chunks per half (8)
    n_mm = min(512, t_half)

    nc = bass.Bass()
    xnat_d = nc.declare_dram_parameter("xnat", [bs // 2, P, tc_n, 2, U], F8, isOutput=False)
    xtr_d = nc.declare_dram_parameter("xtr", [bs, P, EC, te], F8, isOutput=False)
    # Small weights ship pre-permuted and packed so every DMA moves >=512
    # contiguous bytes per partition (tiny strided descriptors -- e.g. va as
    # 256 one-byte RMW descriptors -- took >20us on the HWDGE ring and sat
    # in front of the first tanh's bias).
    uav_d = nc.declare_dram_parameter("uav", [P, EC, U + 16], F8, isOutput=False)
    fsm_d = nc.declare_dram_parameter("fsm", [P, 10, bs], F32, isOutput=False)
    cz_d = nc.declare_dram_parameter("cz", [U, U], BF, isOutput=False)
    cr_d = nc.declare_dram_parameter("cr", [U, U], BF, isOutput=False)
    cp_d = nc.declare_dram_parameter("cp", [U, U], BF, isOutput=False)
    up_d = nc.declare_dram_parameter("up", [U, U], BF, isOutput=False)
    id_d = nc.declare_dram_parameter("ident", [P, P], F32, isOutput=False)
    ht_d = nc.declare_dram_parameter("ht", [bs, U], F32, isOutput=True)

    with tile.TileContext(nc) as tc, ExitStack() as ctx:
        singles = ctx.enter_context(tc.tile_pool(name="singles", bufs=1))
        xnat_p = ctx.enter_context(tc.tile_pool(name="xnat", bufs=2))
        xtr_p = ctx.enter_context(tc.tile_pool(name="xtr", bufs=4))
        tanh_p = ctx.enter_context(tc.tile_pool(name="tanh", bufs=8))
        small_p = ctx.enter_context(tc.tile_pool(name="small", bufs=4))
        uxpb_ps = ctx.enter_context(tc.tile_pool(name="uxpbps", bufs=3, space="PSUM"))
        et_ps = ctx.enter_context(tc.tile_pool(name="etps", bufs=1, space="PSUM"))
        tail_ps = ctx.enter_context(tc.tile_pool(name="tailps", bufs=1, space="PSUM"))

        # ---- weights / small per-core tensors ----
        uav_sb = singles.tile([P, EC, U + 16], F8)  # ua cols 0..255, va col 256
        # (k-tile stride padded to 272 = 17*16: dual-fp8 ldweights needs %16==0)
        fsm_sb = singles.tile([P, 10, bs], F32)     # wxpb 0:2, hT 2:4, g0 4:10
        ua_sb = uav_sb
        va_sb = uav_sb[:, :, U : U + 1]

        def load_first_weights():
            # Head of the sync HWDGE ring: ~150KB, lands ~1us after flow
            # starts, ahead of row 0's x.
            nc.sync.dma_start(out=uav_sb, in_=uav_d[:, :, :])
            nc.sync.dma_start(out=fsm_sb, in_=fsm_d[:, :, :])

        gate_w = {}
        for name in ("cz", "cr", "cp", "up"):
            gate_w[name] = singles.tile([P, EC, U], BF, name=f"{name}_sb")
        id_sb = singles.tile([P, P], F32)
        idb_sb = singles.tile([P, P], BF)
        ones_sb = singles.tile([P, P], F32)
        nc.vector.memset(ones_sb, 1.0)
        ones8 = singles.tile([P, 512], F8)
        nc.vector.memset(ones8, 1.0)

        def load_tail_weights():
            # sync ring, right after row 1: needed from ~iteration 3's ctx.
            nc.sync.dma_start(out=id_sb, in_=id_d[:, :])
            nc.vector.tensor_copy(idb_sb, id_sb)

        def load_gate_weights():
            # sync ring, after pair (6,7)'s xtr (bf16, 128KB each).
            for name, d in (("cz", cz_d), ("cr", cr_d), ("cp", cp_d), ("up", up_d)):
                nc.sync.dma_start(out=gate_w[name], in_=d[:, :].rearrange("(c p) u -> p c u", p=P))

        expsum_all = singles.tile([P, bs], F32)
        # Unnormalized ctx^T columns, staged per pair via tiny PE transposes
        # (no SBUF->SBUF DMA). Normalized in-place per half-batch.
        ctxn = singles.tile([P, EC, bs], BF)
        # Block-diagonal at tiles for the paired-ctx DoubleRow: slot [j, m]
        # holds row (2q+j)'s at iff j == m, else stays the zero written once
        # here. Two tiles ping-pong across pairs. [p, j, tc, m] layout: the
        # k-tile (j) stride is tc_n*2 bytes (dual-fp8 ldweights needs >=16B).
        at2_tiles = []
        for i in range(2):
            at2 = singles.tile([P, 2, tc_n, 2], F8, name=f"at2_{i}")
            nc.vector.memset(at2, 0.0)
            at2_tiles.append(at2)

        # ---- ACT table preload + HAM warmup, both during the initial DMA
        # wait: the weight DMAs issue first on the scalar ring, then a dummy
        # ACT pulls the one-time exp_and_others table load off the
        # first-tanh critical path; ~2.6us of dummy matmuls get the PE
        # clock-gate warming before row 0's data lands.
        load_first_weights()
        actwarm = small_p.tile([P, 1], F32, name="actwarm")
        nc.scalar.activation(out=actwarm, in_=ones_sb[:, 0:1], func=AF.Tanh)
        warm = uxpb_ps.tile([P, 512], F32, tag="ux", name="warm")
        for _ in range(6):
            nc.tensor.matmul(out=warm, lhsT=ones8[:, 0:P], rhs=ones8)

        # ---- streaming stages ----
        pend_pair = {}
        defer_xnat = []

        def stage_dma(b):
            # xtr first in each iteration's ring order: it gates compute one
            # row ahead, while xnat is only needed two iterations later.
            if b < 2:
                # Row 0 in two half-row DMAs (first tanh waits on 256KB);
                # row 1 in one 4KB-descriptor transfer (half-rows have 1KB
                # descriptors, which drain measurably slower).
                xt = singles.tile([P, EC, te], F8, name=f"xt{b}")
                if b == 0:
                    nc.sync.dma_start(out=xt[:, :, 0:t_half], in_=xtr_d[b, :, :, 0:t_half])
                    nc.sync.dma_start(out=xt[:, :, t_half:te], in_=xtr_d[b, :, :, t_half:te])
                else:
                    nc.sync.dma_start(out=xt, in_=xtr_d[b])
            elif b % 2 == 0:
                # One DMA per pair: per-partition-contiguous 2x4KB source
                # blocks -> big descriptors, full HWDGE ring rate.
                xp = xtr_p.tile([P, 2, EC, te], F8, tag="xt", name=f"xt{b}")
                nc.sync.dma_start(
                    out=xp, in_=xtr_d[b : b + 2].rearrange("j p c t -> p j c t")
                )
                pend_pair[b // 2] = xp
                xt = xp[:, 0]
            else:
                xt = pend_pair.pop(b // 2)[:, 1]
            x_nat = None
            if b % 2 == 0:
                x_nat = xnat_p.tile([P, tc_n, 2, U], F8, tag="xnat", name=f"xnat{b}")
                if b == 0:
                    # Deferred behind rows 0/1 + id on the sync ring.
                    defer_xnat.append(x_nat)
                else:
                    # Pair 1 rides the sync ring too: the SDMA engines
                    # round-robin between rings at packet granularity, and
                    # SWDGE's 8KB descriptors would starve the startup-
                    # critical transfers.  From pair 2 on, the xnat pool's
                    # WAR dep (bufs=2) holds the SWDGE ring back until the
                    # matching ctx is done.
                    eng = nc.sync if b == 2 else nc.gpsimd
                    eng.dma_start(out=x_nat, in_=xnat_d[b // 2])
            if b == 1:
                nc.sync.dma_start(out=defer_xnat[0], in_=xnat_d[0])
            return x_nat, xt

        def stage_uxpb_th(b, th, xt):
            # uxpb: out[u, t] = sum_e ua[e, u] * xt[e, t] -- fp8 DoubleRow
            # contracts both e-chunks in one matmul. tanh (per-partition
            # bias) -> SBUF fp8 [u, uc, t] tiles for the et DoubleRow.
            tanh_t = tanh_p.tile([P, UC, t_half], F8, tag="tanh", name=f"th{b}_{th}")
            for uc in range(UC):
                ux = uxpb_ps.tile([P, t_half], F32, tag="ux", name=f"ux{b}{th}{uc}")
                for n0 in range(0, t_half, n_mm):
                    nc.tensor.matmul(
                        out=ux[:, n0 : n0 + n_mm],
                        lhsT=ua_sb[:, :, uc * P : (uc + 1) * P],
                        rhs=xt[:, :, th * t_half + n0 : th * t_half + n0 + n_mm],
                        perf_mode=DR,
                    )
                nc.scalar.activation(
                    out=tanh_t[:, uc, :], in_=ux, func=AF.Tanh,
                    bias=fsm_sb[:, uc, b : b + 1],
                )
            return tanh_t

        def stage_et(b, tanh_ts, et_pair):
            j = b % 2
            for th in range(th_n):
                for tq in range(tq_n):
                    nc.tensor.matmul(
                        out=et_pair[:, j, th * tq_n + tq : th * tq_n + tq + 1],
                        lhsT=tanh_ts[th][:, :, tq * P : (tq + 1) * P],
                        rhs=va_sb,
                        perf_mode=DR,
                    )

        def stage_exp_pair(q, et_pair):
            # One exp ACT for the whole pair -> fp8 staging; DVE lands the
            # two rows on the block-diag at2 diagonal slots and reduces the
            # staging for expsum (no ACT accumulator reads).
            at2 = at2_tiles[q % 2]
            exps = small_p.tile([P, 2, tc_n], F8, tag="exps", name=f"exps{q}")
            nc.scalar.activation(out=exps, in_=et_pair, func=AF.Exp)
            for j in range(2):
                nc.vector.tensor_copy(at2[:, j, :, j], exps[:, j, :])
            nc.vector.tensor_reduce(
                out=expsum_all[:, 2 * q : 2 * q + 2], in_=exps,
                axis=AX.X, op=ALU.add,
            )
            return at2

        def stage_ctx_half(q, at2, x_nat, cps, lo, hi):
            # Paired ctx: block-diagonal at2 on the two k-tiles against the
            # pair-interleaved x tile -> out[m, e] = row (2q+m)'s ctx partial.
            # Issued as two 8-matmul chunks in consecutive iterations so PE
            # load stays balanced against the ACT tanh pace.
            if cps is None:
                cps = tail_ps.tile([2, U], F32, tag="tail", name=f"cps{q}")
            for tcc in range(lo, hi):
                nc.tensor.matmul(
                    out=cps,
                    lhsT=at2[:, :, tcc, :],
                    rhs=x_nat[:, tcc, :, :],
                    start=(tcc == 0),
                    stop=(tcc == tc_n - 1),
                    perf_mode=DR,
                )
            return cps

        def stage_ctx_finish(q, cps):
            # Stage straight into ctxn columns via tiny PE transposes (bf16),
            # unnormalized; 1/s is applied once per half-batch.
            stg = small_p.tile([2, U], BF, tag="ctxstg", name=f"stg{q}")
            nc.vector.tensor_copy(stg, cps)
            for e in range(EC):
                tp = tail_ps.tile([P, 2], BF, tag="tail", name=f"ctxT{q}{e}")
                nc.tensor.transpose(tp, stg[:, e * P : (e + 1) * P], idb_sb[0:2, 0:2])
                nc.vector.tensor_copy(ctxn[:, e, 2 * q : 2 * q + 2], tp)

        # ---- tail: gates + output, per row-group.  Group 0 (rows 0-7) is
        # spread one small stage per iteration inside the stream (exp-based
        # sigmoid, same ACT table set).  Groups 1 (rows 8-13) and 2 (rows
        # 14-15) run in the epilogue with the real Sigmoid table (exp is
        # never needed again; the table switch hides under pair-7's ctx
        # matmuls).  Group 1 depends only on pairs 4-6, so its whole chain
        # overlaps group 2's wait on pair-7.
        GRP = {0: (0, 8), 1: (8, 14), 2: (14, 16)}
        recips_g = {}
        zt_g = {}
        rh_g = {}

        def tail_s(g):
            lo, hi = GRP[g]
            s_ps = tail_ps.tile([P, hi - lo], F32, tag="tail", name=f"sps{g}")
            nc.tensor.matmul(out=s_ps, lhsT=ones_sb, rhs=expsum_all[:, lo:hi])
            rec = small_p.tile([P, hi - lo], F32, name=f"recip{g}")
            nc.vector.reciprocal(rec, s_ps)
            recips_g[g] = rec

        def tail_mul(g):
            lo, hi = GRP[g]
            for e in range(EC):
                nc.vector.tensor_mul(ctxn[:, e, lo:hi], ctxn[:, e, lo:hi], recips_g[g])

        def _gate_psum(g, parts, name):
            # One [P, UC, n] psum tile accumulating all (weight, rhs) pairs
            # for both u-chunks -> a single ACT covers the whole gate.
            lo, hi = GRP[g]
            gp = tail_ps.tile([P, UC, hi - lo], F32, tag="tail", name=name)
            for uc in range(UC):
                i = 0
                for w_sb, rhs_fn in parts:
                    for e in range(EC):
                        nc.tensor.matmul(
                            out=gp[:, uc, :],
                            lhsT=w_sb[:, e, uc * P : (uc + 1) * P],
                            rhs=rhs_fn(e),
                            start=(i == 0),
                            stop=(i == len(parts) * EC - 1),
                        )
                        i += 1
            return gp

        def _tail_gate(g, gi, wname, sigm):
            # One gate chain: MMs -> +g0 -> sigmoid -> (rh mul).
            lo, hi = GRP[g]
            n = hi - lo
            gp = _gate_psum(g, [(gate_w[wname], lambda e: ctxn[:, e, lo:hi])], f"g{wname}{g}")
            tmp = small_p.tile([P, UC, n], F32, tag="gtmp", name=f"t{wname}{g}")
            nc.vector.tensor_add(tmp, gp, fsm_sb[:, 4 + 2 * gi : 6 + 2 * gi, lo:hi])
            dst = small_p.tile([P, UC, n], F32, name=f"sg{wname}{g}")
            if sigm:
                nc.scalar.activation(out=dst, in_=tmp, func=AF.Sigmoid)
            else:
                # in-stream: sigmoid(v) = 1/(1+exp(-v)) keeps the exp table
                ex = small_p.tile([P, UC, n], F32, tag="gtmp", name=f"e{wname}{g}")
                nc.scalar.activation(out=ex, in_=tmp, func=AF.Exp, scale=-1.0)
                nc.vector.tensor_scalar_add(ex, ex, 1.0)
                nc.vector.reciprocal(dst, ex)
            if gi == 0:
                zt_g[g] = dst
            else:
                rh = small_p.tile([P, UC, n], BF, name=f"rh{g}")
                nc.vector.tensor_mul(rh, dst, fsm_sb[:, 2:4, lo:hi])
                rh_g[g] = rh

        def tail_z(g, sigm=False):
            _tail_gate(g, 0, "cz", sigm)

        def tail_r(g, sigm=False):
            _tail_gate(g, 1, "cr", sigm)

        def tail_p_out(g):
            lo, hi = GRP[g]
            n = hi - lo
            zt, rh = zt_g[g], rh_g[g]
            gp = _gate_psum(
                g,
                [(gate_w["up"], lambda e: rh[:, e, :]),
                 (gate_w["cp"], lambda e: ctxn[:, e, lo:hi])],
                f"gp{g}",
            )
            gtmp = small_p.tile([P, UC, n], F32, tag="gtmp", name=f"gt{g}")
            nc.vector.tensor_add(gtmp, gp, fsm_sb[:, 8:10, lo:hi])
            tht = small_p.tile([P, UC, n], F32, tag="gtmp", name=f"tht{g}")
            nc.scalar.activation(out=tht, in_=gtmp, func=AF.Tanh)
            # ht^T = h^T + zt^T*(tht^T - h^T)
            nc.vector.tensor_sub(tht, tht, fsm_sb[:, 2:4, lo:hi])
            nc.vector.tensor_mul(tht, tht, zt)
            nc.vector.tensor_add(tht, tht, fsm_sb[:, 2:4, lo:hi])
            stg = small_p.tile([n, U], F32, name=f"htstg{g}")
            for uc in range(UC):
                tp = tail_ps.tile([n, P], F32, tag="tail", name=f"htp{g}{uc}")
                nc.tensor.transpose(tp, tht[:, uc, :], id_sb)
                nc.vector.tensor_copy(stg[:, uc * P : (uc + 1) * P], tp)
            nc.sync.dma_start(out=ht_d[lo:hi, :], in_=stg)

        def keepalive(i):
            # One matmul to reset the PE clock-gate's idle window during the
            # latency-bound tail (else it re-throttles to 1.2GHz mid-tail).
            ka = uxpb_ps.tile([P, 512], F32, tag="ux", name=f"ka{i}")
            nc.tensor.matmul(out=ka, lhsT=ones8[:, 0:P], rhs=ones8)

        # ---- main loop, software-pipelined one row deep ----
        prev = None  # (b, tanh_ts)
        pair_xnat = {}
        pend_ctx = None  # (q, cps) with 8 of 16 matmuls issued
        et_pair = None
        at2_prev = None
        for b in range(bs):
            x_nat, xt = stage_dma(b)
            if x_nat is not None:
                pair_xnat[b // 2] = x_nat
            if b == 1:
                load_tail_weights()
            if b == 6:
                load_gate_weights()
            th0 = stage_uxpb_th(b, 0, xt)
            # Tail work for rows 0-7, spread one small stage per iteration,
            # placed right after th0 so its tiny matmuls complete well
            # before its ACT slot comes up (no ACT head-of-line stall).
            if b == 11:
                tail_s(0)
                tail_mul(0)
            elif b == 12:
                tail_z(0)
            elif b == 13:
                tail_r(0)
            elif b == 14:
                tail_p_out(0)
            if pend_ctx is not None:
                q, cps = pend_ctx
                stage_ctx_finish(q, stage_ctx_half(q, at2_tiles[q % 2], pair_xnat.pop(q), cps, tc_n // 2, tc_n))
                pend_ctx = None
            if prev is not None:
                pb = prev[0]
                if pb % 2 == 0:
                    et_pair = et_ps.tile([P, 2, tc_n], F32, tag="etp", name=f"et{pb}")
                stage_et(pb, prev[1], et_pair)
                if pb % 2 == 1:
                    at2_prev = stage_exp_pair(pb // 2, et_pair)
            th1 = stage_uxpb_th(b, 1, xt)
            if prev is not None and pb % 2 == 1 and pb < bs - 1:
                q = pb // 2
                pend_ctx = (q, stage_ctx_half(q, at2_prev, pair_xnat[q], None, 0, tc_n // 2))
            prev = (b, [th0, th1])
        stage_et(prev[0], prev[1], et_pair)
        keepalive(0)
        at2_last = stage_exp_pair(prev[0] // 2, et_pair)
        tail_s(1)
        tail_s(2)
        # Switch the ACT table set to sigmoid_and_others (also has tanh);
        # the ~2.7us load hides under pair-7's ctx matmuls.
        sigwarm = small_p.tile([P, 1], F32, name="sigwarm")
        nc.scalar.activation(out=sigwarm, in_=ones_sb[:, 0:1], func=AF.Sigmoid)
        # Group 1 (rows 8-13) has no dependency on pair 7: its small gate
        # matmuls go FIRST in the PE FIFO so its ACT/DVE chain runs while
        # pair-7's 16 ctx matmuls stream right behind them.
        tail_mul(1)
        tail_z(1, sigm=True)
        tail_r(1, sigm=True)
        q = prev[0] // 2
        stage_ctx_finish(q, stage_ctx_half(q, at2_last, pair_xnat.pop(q), None, 0, tc_n))
        keepalive(1)
        tail_p_out(1)
        tail_mul(2)
        tail_z(2, sigm=True)
        tail_r(2, sigm=True)
        keepalive(2)
        tail_p_out(2)

    if split_waits:
        split_multi_waits(nc)
    return nc


def _host_prep(inputs, h_tm, V_a, W_a, U_a, b_a, C_z, W_z, b_z, C_r, W_r, b_r,
               C_p, U_p, b_p):
    """Fold everything not depending on x_seq into small per-core tensors."""
    wxpb = h_tm @ W_a + b_a                                # [B, U]
    g_z0 = h_tm @ W_z + inputs @ C_z[:IN_DIM] + b_z        # [B, U]
    g_r0 = h_tm @ W_r + inputs @ C_r[:IN_DIM] + b_r
    g_p0 = inputs @ C_p[:IN_DIM] + b_p
    # uav[p, c, :]: ua row (c*128+p) in cols 0..U-1, va[c*128+p] in col U --
    # one packed DMA with >=512 contiguous bytes per partition.
    uav = np.zeros((P, EC, U + 16), dtype=NPF8)
    uav[:, :, :U] = U_a.astype(NPF8).reshape(EC, P, U).transpose(1, 0, 2)
    uav[:, :, U] = V_a.astype(NPF8).reshape(EC, P).T
    shared = {
        "uav": np.ascontiguousarray(uav),
        "cz": np.ascontiguousarray(C_z[IN_DIM:].astype(BF16)),
        "cr": np.ascontiguousarray(C_r[IN_DIM:].astype(BF16)),
        "cp": np.ascontiguousarray(C_p[IN_DIM:].astype(BF16)),
        "up": np.ascontiguousarray(U_p.astype(BF16)),
        "ident": np.eye(P, dtype=np.float32),
    }
    per_core = []
    for c in range(N_CORES):
        s = slice(c * BS, (c + 1) * BS)
        # fsm[p, 0:2]: wxpb^T; [p, 2:4]: h^T; [p, 4:10]: g0^T for z, r, p --
        # all [u%128 -> p, u//128 -> chunk, b] layouts packed in one tensor.
        def chunked(m):  # [bs, U] -> [P, UC, bs]
            return m.T.astype(np.float32).reshape(UC, P, BS).transpose(1, 0, 2)
        fsm = np.concatenate(
            [chunked(wxpb[s]), chunked(h_tm[s]), chunked(g_z0[s]),
             chunked(g_r0[s]), chunked(g_p0[s])], axis=1
        )
        per_core.append({"fsm": np.ascontiguousarray(fsm), **shared})
    return per_core


def _prep_x(x_core):
    """Pre-tile one core's x [bs, TE, U] into both fp8 layouts."""
    xb = x_core.astype(NPF8)
    tc_n = TE // P
    # xnat[q, p, tc, j, e] = x[2q+j, tc*128+p, e]  (pair-interleaved)
    xnat = np.ascontiguousarray(
        xb.reshape(BS // 2, 2, tc_n, P, U).transpose(0, 3, 2, 1, 4)
    )
    # xtr[b, p, ec, t] = x[b, t, ec*128+p]
    xtr = np.ascontiguousarray(
        xb.reshape(BS, TE, EC, P).transpose(0, 3, 2, 1)
    )
    return xnat, xtr


def build_in_maps(all_inputs):
    """Full host prep: dict of the reference's 16 inputs -> per-core in_maps."""
    args = {k: np.asarray(v, dtype=np.float32) for k, v in all_inputs.items()
            if k != "x_seq"}
    x_seq = np.asarray(all_inputs["x_seq"], dtype=np.float32)
    per_core = _host_prep(**args)
    in_maps = []
    for c in range(N_CORES):
        m = dict(per_core[c])
        m["xnat"], m["xtr"] = _prep_x(x_seq[c * BS : (c + 1) * BS])
        in_maps.append(m)
    return in_maps


def kernel(inputs, h_tm, x_seq, V_a, W_a, U_a, b_a, C_z, W_z, b_z,
           C_r, W_r, b_r, C_p, U_p, b_p):
    from concourse.bass_utils import run_bass_kernel_spmd

    in_maps = build_in_maps(dict(
        inputs=inputs, h_tm=h_tm, x_seq=x_seq, V_a=V_a, W_a=W_a, U_a=U_a,
        b_a=b_a, C_z=C_z, W_z=W_z, b_z=b_z, C_r=C_r, W_r=W_r, b_r=b_r,
        C_p=C_p, U_p=U_p, b_p=b_p))
    nc = build_nc()
    res = run_bass_kernel_spmd(nc, in_maps, core_ids=list(range(N_CORES)))
    return np.concatenate([res.results[c]["ht"] for c in range(N_CORES)], axis=0)
